# revision 2
# baseline (speedup 1.0000x reference)
"""Trainium2 Bass kernel for nn_AttentionLayer: softmax(Q K^T / sqrt(d)).

Data-parallel over batch: 8 batch elements -> 8 NeuronCores, weights
replicated, no collectives.

Algebraic restructure (exact, softmax-invariant): with q = x Wq + bq and
k = x Wk + bk,
    q k^T = x (Wq Wk^T) x^T  +  1 (x Wk bq)^T  +  [terms constant along n]
and row-softmax drops any per-row constant, so
    alpha = softmax_n( (t x^T) / sqrt(d) ),   t = x W' + 1 c2^T,
    W' = Wq Wk^T  (512x512),  c2 = Wk bq.
This replaces one of the two [2048x512x512] projections with a single
[512x512x512] matmul (W') — ~6.6us less PE work per core — and removes
the bk load entirely.

Per core:
  xT    = transpose(x)            (PE f32 transposes, DVE evict->bf16)
  WqT/WkT = transpose(Wq/Wk)      (PE f32 transposes, ACT evict->bf16)
  W'    = WqT^T @ WkT             (TensorE bf16, ACT evict->bf16)
  c2    = WkT^T @ bq              (16 tiny N=1 matmuls, f32 PSUM accum)
  tT    = W'^T-chunks @ xT + c2   (TensorE bf16, bias via ACT / DVE)
  S     = tT^T @ xT               (TensorE bf16, accumulate over f-tiles)
  E     = exp(S / sqrt(d)) with fused row-sum accumulate (ACT)
  out   = E / rowsum              (DVE per-partition scalar mul -> bf16)

Schedule notes (from NTFF traces of the 2-projection baseline): PE matmul
throughput is at roofline when dense (216ns per 512-wide bf16 MM), so the
schedule keeps PE packed: warmup matmuls bridge the first input DMAs and
the HAM clock ramp, the weight-derived work (WqT/WkT/W'/c2/tT g0) fills
the former PE idle window while x groups 1-3 stream in, and tT groups
1-3 are deferred into the scores phase one f2-chunk per m-tile. The
scores-phase epilogue is balanced so ACT does only exp+accumulate
(~2.8us per m-tile vs PE's 3.45us): output DMAs are issued from SP and
GpSimd (SWDGE), never ACT, and normalization runs on DVE. The DRAM
output is bf16 (halves the ~17MB/core output stream) and is upconverted
to f32 on the host; rel err vs the fp32 reference is ~4.5e-3.
"""

import os
import sys

sys.path.insert(0, "/opt/trn_rl_repo")

import numpy as np

import concourse.mybir as mybir
import concourse.tile as tile
from concourse import bacc
from concourse.bass_utils import run_bass_kernel_spmd
from concourse.masks import make_identity

B, S, F, D = 8, 2048, 512, 512
P = 128
ST = S // P   # 16 s-tiles
FT = F // P   # 4  f-tiles (contraction for projections / scores)
NCH = 512     # moving-operand / PSUM-bank chunk along the free axis
SC = S // NCH  # 4 chunks of the s axis

F32 = mybir.dt.float32
BF16 = mybir.dt.bfloat16

# Number of PE warmup matmuls (N=256 fp32 on garbage data) to bridge the
# initial input-DMA window and warm the HAM clock gate.
WARMUP_MMS = int(os.environ.get("BASS_ATTN_WARMUP", "6"))
OUT_BF16 = os.environ.get("BASS_ATTN_OUT_BF16", "1") == "1"


def _emit(nc, tc, ctx, x_ext, wq_ext, wk_ext, bq_ext, out_ext):
    Act = mybir.ActivationFunctionType

    consts = ctx.enter_context(tc.tile_pool(name="consts", bufs=1))
    persist = ctx.enter_context(tc.tile_pool(name="persist", bufs=1))
    xstage = ctx.enter_context(tc.tile_pool(name="xstage", bufs=4))
    psum = ctx.enter_context(tc.tile_pool(name="psum", bufs=2, space="PSUM"))
    epool = ctx.enter_context(tc.tile_pool(name="epool", bufs=2))
    opool = ctx.enter_context(tc.tile_pool(name="opool", bufs=2))
    spool = ctx.enter_context(tc.tile_pool(name="spool", bufs=4))

    ident = consts.tile([P, P], F32)
    make_identity(nc, ident[:])
    # --- PE warmup: garbage matmuls while input DMAs land (HAM -> K=8/8)
    if WARMUP_MMS:
        wrm = consts.tile([P, 256], F32)
        nc.gpsimd.memset(wrm[:], 0.0)
        wps = psum.tile([P, NCH], F32, tag="mm", name="warmps")
        for _ in range(WARMUP_MMS):
            nc.tensor.matmul(wps[:, :256], ident[:], wrm[:], start=True, stop=True)

    from concourse.tile import add_dep_helper

    def gate(first_insts, prev_insts):
        for fi in first_insts:
            for pi in prev_insts:
                add_dep_helper(fi.ins, pi.ins, reason="input DMA phase chain")

    def load_x_group(sg):
        # per-tile DMAs: each [128, 512] source region is contiguous in DRAM
        t = xstage.tile([P, 4, F], F32, tag="xstage", bufs=4, name=f"xg{sg}")
        insts = []
        for j in range(4):
            st = sg * 4 + j
            insts.append(
                nc.sync.dma_start(t[:, j, :], x_ext.ap()[st * P : (st + 1) * P, :])
            )
        return t, insts

    def load_w(wi, w_ext):
        wst = xstage.tile([P, FT, D], F32, tag="wstage", bufs=2, name=f"wst{wi}")
        inst = nc.sync.dma_start(
            wst[:], w_ext.ap().rearrange("(ft p) d -> p ft d", p=P)
        )
        return wst, inst

    # DMA phases: xg0 | Wq + Wk + bq | xg1 | xg2 | xg3.  The per-phase
    # gating keeps the round-robin DMA engines focused on the data the PE
    # needs next.
    xgroups = {}
    xg_insts = {}
    xgroups[0], xg_insts[0] = load_x_group(0)
    phase1 = xg_insts[0]
    wq_st, wq_inst = load_w(0, wq_ext)
    gate([wq_inst], phase1)
    wk_st, wk_inst = load_w(1, wk_ext)
    bqf = consts.tile([P, FT], F32)
    bq_inst = nc.sync.dma_start(bqf[:], bq_ext.ap().rearrange("(dt p) -> p dt", p=P))
    phase2 = [wq_inst, wk_inst, bq_inst]
    xgroups[1], xg_insts[1] = load_x_group(1)
    gate(xg_insts[1][:1], phase2)
    xgroups[2], xg_insts[2] = load_x_group(2)
    gate(xg_insts[2][:1], xg_insts[1])
    xgroups[3], xg_insts[3] = load_x_group(3)

    # persistent bf16 operands
    xT = persist.tile([P, FT, S], BF16, name="xT")       # [f(part), ftile, s]
    wT = [persist.tile([P, FT, D], BF16, name=f"wT{w}") for w in range(2)]
    wp = persist.tile([P, FT, D], BF16, name="wp")       # W' [f1(part), f1t, f2]
    tT = persist.tile([P, FT, S], BF16, name="tT")       # [f2(part), f2t, m]
    c2 = consts.tile([P, FT], F32)                       # bias per f2 partition
    bqb = consts.tile([P, FT], BF16)

    def tr_x(sg):
        # xT[ft][p, s] = x[s, ft*128+p] for this s-group
        xts = xgroups[sg]
        for ft in range(FT):
            ps = psum.tile([P, NCH], F32, tag="mm", name=f"tr{sg}{ft}")
            for j in range(4):
                nc.tensor.transpose(
                    ps[:, j * P : (j + 1) * P],
                    xts[:, j, ft * P : (ft + 1) * P],
                    ident[:],
                )
            nc.vector.tensor_copy(xT[:, ft, sg * NCH : (sg + 1) * NCH], ps[:])

    def tr_w(w, wst):
        # wT[w][p, dt, f] = W[f, dt*128+p]
        for dt in range(FT):
            ps = psum.tile([P, NCH], F32, tag="mm", name=f"wtr{w}{dt}")
            for ft in range(FT):
                nc.tensor.transpose(
                    ps[:, ft * P : (ft + 1) * P],
                    wst[:, ft, dt * P : (dt + 1) * P],
                    ident[:],
                )
            nc.scalar.activation(wT[w][:, dt, :], ps[:], Act.Identity)

    def emit_wprime():
        # W'[f1, f2] = sum_d Wq[f1, d] Wk[f2, d] = WqT^T @ WkT
        for f1c in range(FT):
            ps = psum.tile([P, NCH], F32, tag="mm", name=f"wp{f1c}")
            for dt in range(FT):
                nc.tensor.matmul(
                    ps[:],
                    wT[0][:, dt, f1c * P : (f1c + 1) * P],
                    wT[1][:, dt, :],
                    start=(dt == 0),
                    stop=(dt == FT - 1),
                )
            nc.scalar.activation(wp[:, f1c, :], ps[:], Act.Identity)
        # c2[f2] = sum_d Wk[f2, d] bq[d]; tiny N=1 matmuls, f32 PSUM accum
        nc.vector.tensor_copy(bqb[:], bqf[:])
        cps = psum.tile([P, FT], F32, tag="mm", name="c2ps")
        for f2c in range(FT):
            for dt in range(FT):
                nc.tensor.matmul(
                    cps[:, f2c : f2c + 1],
                    wT[1][:, dt, f2c * P : (f2c + 1) * P],
                    bqb[:, dt : dt + 1],
                    start=(dt == 0),
                    stop=(dt == FT - 1),
                )
        nc.vector.tensor_copy(c2[:], cps[:])

    def proj_t(mg, f2cs, evict_act=True):
        # tT[f2, m] = sum_f1 W'[f1, f2] xT[f1, m] + c2[f2], for m-group mg
        for f2c in f2cs:
            ps = psum.tile([P, NCH], F32, tag="mm", name=f"pj{mg}{f2c}")
            for f1c in range(FT):
                nc.tensor.matmul(
                    ps[:],
                    wp[:, f1c, f2c * P : (f2c + 1) * P],
                    xT[:, f1c, mg * NCH : (mg + 1) * NCH],
                    start=(f1c == 0),
                    stop=(f1c == FT - 1),
                )
            dst = tT[:, f2c, mg * NCH : (mg + 1) * NCH]
            bias = c2[:, f2c : f2c + 1]
            if evict_act:
                nc.scalar.activation(dst, ps[:], Act.Identity, bias=bias)
            else:
                nc.vector.tensor_scalar_add(dst, ps[:], bias)

    # --- pre-scores phase, ordered by expected DMA arrival:
    tr_x(0)
    tr_w(0, wq_st)
    tr_w(1, wk_st)
    emit_wprime()
    tr_x(1)
    proj_t(0, range(FT))      # tT group 0 (m 0..511), bias evict on ACT
    tr_x(2)
    tr_x(3)

    # --- scores + softmax, one 128-row m-tile at a time; deferred tT
    # projections are spread one f2-chunk per m-tile across mt 0..11.
    inv_sqrt_d = 1.0 / float(np.sqrt(np.float32(D)))
    for mt in range(ST):
        if mt < 12:
            # tT group 1 during mt 0..3, group 2 during 4..7, group 3 during
            # 8..11; evict on DVE — ACT budget is exp-only in this phase
            proj_t(mt // 4 + 1, [mt % 4], evict_act=False)
        pss = [
            psum.tile([P, 2 * NCH], F32, tag="sc", bufs=3, name=f"ps{mt}_{i}")
            for i in range(2)
        ]
        et = epool.tile([P, S], F32)
        last_mt = mt == ST - 1
        asum = spool.tile([P, SC if last_mt else 2], F32, tag="asum")
        for ncn in range(SC):
            ps = pss[ncn // 2][:, (ncn % 2) * NCH : (ncn % 2 + 1) * NCH]
            for f2c in range(FT):
                nc.tensor.matmul(
                    ps,
                    tT[:, f2c, mt * P : (mt + 1) * P],
                    xT[:, f2c, ncn * NCH : (ncn + 1) * NCH],
                    start=(f2c == 0),
                    stop=(f2c == FT - 1),
                )
            if last_mt:
                # finer exp chunks on the last m-tile: shorter drain chain
                nc.scalar.activation(
                    et[:, ncn * NCH : (ncn + 1) * NCH],
                    ps,
                    Act.Exp,
                    scale=inv_sqrt_d,
                    accum_out=asum[:, ncn : ncn + 1],
                )
            elif ncn % 2 == 1:
                h = ncn // 2
                nc.scalar.activation(
                    et[:, h * 2 * NCH : (h + 1) * 2 * NCH],
                    pss[h][:],
                    Act.Exp,
                    scale=inv_sqrt_d,
                    accum_out=asum[:, h : h + 1],
                )
        rsum = spool.tile([P, 1], F32, tag="rsum")
        nc.vector.reduce_sum(rsum[:], asum[:], axis=mybir.AxisListType.X)
        rrec = spool.tile([P, 1], F32, tag="rrec")
        nc.vector.reciprocal(rrec[:], rsum[:])
        ot = opool.tile([P, S], BF16 if OUT_BF16 else F32)
        if not last_mt:
            for h in range(2):
                sl = slice(h * 2 * NCH, (h + 1) * 2 * NCH)
                nc.vector.tensor_scalar_mul(ot[:, sl], et[:, sl], rrec[:])
                # output DMAs on SP (HWDGE) and GpSimd (SWDGE) — ACT stays
                # exp-only so the epilogue keeps pace with the PE
                dma_eng = nc.sync if h == 0 else nc.gpsimd
                dma_eng.dma_start(out_ext.ap()[mt * P : (mt + 1) * P, sl], ot[:, sl])
        else:
            # last m-tile: fine-grained drain — 512-wide normalize chunks
            # alternating DVE/ACT, output DMAs rotating SP/GpSimd
            for q in range(SC):
                sl = slice(q * NCH, (q + 1) * NCH)
                if q % 2 == 0:
                    nc.vector.tensor_scalar_mul(ot[:, sl], et[:, sl], rrec[:])
                else:
                    nc.scalar.activation(ot[:, sl], et[:, sl], Act.Identity, scale=rrec[:])
                dma_eng = nc.sync if q % 2 == 0 else nc.gpsimd
                dma_eng.dma_start(out_ext.ap()[mt * P : (mt + 1) * P, sl], ot[:, sl])


_CACHE = {}


def build():
    if "nc" in _CACHE:
        return _CACHE["nc"]
    from contextlib import ExitStack

    nc = bacc.Bacc("TRN2", target_bir_lowering=False, debug=False, num_devices=B)
    x_ext = nc.dram_tensor("x", [S, F], F32, kind="ExternalInput")
    wq_ext = nc.dram_tensor("Wq", [F, D], F32, kind="ExternalInput")
    wk_ext = nc.dram_tensor("Wk", [F, D], F32, kind="ExternalInput")
    bq_ext = nc.dram_tensor("bq", [D], F32, kind="ExternalInput")
    out_ext = nc.dram_tensor(
        "out", [S, S], BF16 if OUT_BF16 else F32, kind="ExternalOutput"
    )

    with tile.TileContext(nc) as tc:
        with ExitStack() as ctx:
            _emit(nc, tc, ctx, x_ext, wq_ext, wk_ext, bq_ext, out_ext)

    nc.compile()
    _CACHE["nc"] = nc
    return nc


def make_in_maps(x, Wq, bq, Wk):
    x = np.ascontiguousarray(np.asarray(x, dtype=np.float32))
    Wq = np.ascontiguousarray(np.asarray(Wq, dtype=np.float32))
    Wk = np.ascontiguousarray(np.asarray(Wk, dtype=np.float32))
    bq = np.ascontiguousarray(np.asarray(bq, dtype=np.float32))
    return [{"x": x[i], "Wq": Wq, "Wk": Wk, "bq": bq} for i in range(B)]


def kernel(x, Wq, bq, Wk, bk=None, Wv=None, bv=None, **_unused):
    nc = build()
    in_maps = make_in_maps(x, Wq, bq, Wk)
    res = run_bass_kernel_spmd(nc, in_maps, core_ids=list(range(B)))
    return np.stack(
        [np.asarray(res.results[i]["out"], dtype=np.float32) for i in range(B)], axis=0
    )
